# revision 19
# baseline (speedup 1.0000x reference)
"""AmplitudeWeightedPhaseAttention Trainium2 kernel (8 NeuronCores, SPMD).

Raw Block implementation with sequence-parallel feature computation:
each core computes rfft features only for its own 256 K rows and
all-gathers them across its 4-core batch group (flash-style), cutting the
ACT/DVE elementwise work 2.5x versus replicating full-K features.

Math: with rfft bin features re/im and amp2 = re^2 + im^2:
    t  = amp2^(-1/4)       u = re*t   v = im*t   w = amp2^(+1/4)
    num[i,j] = sum_f u_q u_k + v_q v_k        (v==0 at f=0 and f=64)
    den[i,j] = sum_f w_q w_k
    weights  = softmax_j(num/den + 1)         out = weights @ V
Scores in natural [i_p, j] layout (softmax norm = per-partition scalar);
normalized weights are PE-transposed to feed P@V.

PSUM bank plan (stack-scoped, aliasing ordered by semaphore ledger):
  phase1: qt[0] kt[1]   phase2: re[0] im[1]
  phase3: sc0[0-1] sc1[2-3] wt0[4] wt1[5] av[6]
"""

import numpy as np

B, S, H = 2, 1024, 128
F = H // 2 + 1  # 65
NCORES = 8
QBLK = S // 4
NJT = S // 128
NIT = QBLK // 128
NFO = 2 * QBLK  # own feature cols: Q [0:256], K-own [256:512]

WEIGHTS_BF16 = True

_CACHE = {}


def _dft_consts():
    h = np.arange(H, dtype=np.float64)[:, None]
    f = np.arange(F, dtype=np.float64)[None, :]
    C = np.cos(2 * np.pi * h * f / H)
    Sn = -np.sin(2 * np.pi * h * f / H)
    Sn[:, 0] = 0.0
    Sn[:, F - 1] = 0.0
    return np.concatenate([C, Sn], axis=1).astype(np.float32)  # [128, 130]


def _patch_act_tables():
    """Make Ln and Exp both first-match to natural_log_exp_and_others so one
    ACT table load covers the whole kernel (set IDs stay canonical)."""
    import concourse.bacc as bacc_mod
    from concourse import mybir
    if getattr(bacc_mod, "_awpa_tables_patched", False):
        return
    orig = bacc_mod.get_activation_tables
    AF = mybir.ActivationFunctionType

    def patched(arch):
        tables = dict(orig(arch))
        if "natural_log_exp_and_others" in tables:
            for name, fns in tables.items():
                if name != "natural_log_exp_and_others" and \
                        (AF.Ln in fns or AF.Exp in fns):
                    tables[name] = fns - {AF.Ln, AF.Exp}
        return tables

    bacc_mod.get_activation_tables = patched
    bacc_mod._awpa_tables_patched = True


class Led:
    """Pre-declared cumulative semaphore ledger."""

    def __init__(self, sems, orders):
        self.sems = sems
        self.ev = {}
        self.next = {}
        self.plan = {}
        for s, seq in orders.items():
            c = 0
            self.plan[s] = list(seq)
            for name, by in seq:
                c += by
                self.ev[name] = (s, c)
            self.next[s] = 0

    def inc(self, instr, s, name):
        want, by = self.plan[s][self.next[s]]
        assert want == name, f"sem {s}: expected {want}, got {name}"
        self.next[s] += 1
        instr.then_inc(self.sems[s], by)

    def wait(self, eng, name):
        s, n = self.ev[name]
        eng.wait_ge(self.sems[s], n)


def _build():
    import contextlib
    import concourse.bass as bass
    from concourse import bacc, mybir

    _patch_act_tables()
    f32 = mybir.dt.float32
    bf16 = mybir.dt.bfloat16
    AF = mybir.ActivationFunctionType

    nc = bacc.Bacc("TRN2", target_bir_lowering=False, debug=False,
                   num_devices=NCORES)
    Qs = nc.dram_tensor("Qs", [QBLK, H], f32, kind="ExternalInput").ap()
    Ks = nc.dram_tensor("Ks", [QBLK, H], f32, kind="ExternalInput").ap()
    Vd = nc.dram_tensor("V", [S, H], f32, kind="ExternalInput").ap()
    IDTCD = nc.inline_tensor(np.concatenate(
        [np.eye(H, dtype=np.float32), _dft_consts()], axis=1), "IDTCD").ap()
    ow_dt = bf16 if WEIGHTS_BF16 else f32
    OW = nc.dram_tensor("OW", [QBLK, S], ow_dt, kind="ExternalOutput").ap()
    OO = nc.dram_tensor("OO", [H, QBLK], f32, kind="ExternalOutput").ap()
    # collective bounce buffers (u | v | w stacked on rows)
    NR = 3 * F - 1  # 194 rows
    b_all = nc.dram_tensor("b_all", [NR, QBLK], bf16).ap()
    g_all = nc.dram_tensor("g_all", [4, NR, QBLK], bf16).ap()

    def sbuf(name, shape, dt):
        return nc.alloc_sbuf_tensor(name, list(shape), dt).ap()

    junk = sbuf("junk", [128, 1], f32)
    junk2 = sbuf("junk2", [128, 1], f32)
    idtcd = sbuf("idtcd", [H, H + 2 * F], f32)
    idt = idtcd[:, 0:H]
    cdf = idtcd[:, H:H + 2 * F]
    idtb = sbuf("idtb", [H, H], bf16)
    cdb = sbuf("cdb", [H, 2 * F], bf16)
    ksn = sbuf("ksn", [128, NIT, H], f32)
    qn = sbuf("qn", [128, NIT, H], f32)
    vn = sbuf("vn", [128, NJT, H], f32)
    vb = sbuf("vb", [128, NJT, H], bf16)
    xtb = sbuf("xtb", [128, NFO], bf16)   # transposed [Q | K-own]
    uS = sbuf("uS", [F, NFO], bf16)       # own features (Q + K-own)
    vS = sbuf("vS", [F - 1, NFO], bf16)
    wS = sbuf("wS", [F, NFO], bf16)
    uF = sbuf("uF", [F, 4, QBLK], bf16)   # gathered K features, all 1024 j
    vF = sbuf("vF", [F - 1, 4, QBLK], bf16)
    wF = sbuf("wF", [F, 4, QBLK], bf16)
    sqre = sbuf("sqre", [F, NFO], bf16)
    sqim = sbuf("sqim", [F, NFO], bf16)
    amp2 = sbuf("amp2", [F, NFO], bf16)
    lg = sbuf("lg", [F, NFO], f32)
    tq = sbuf("tq", [F, NFO], f32)
    inv = sbuf("inv", [128, 512], f32)
    pa = [sbuf(f"pa{i}", [128, 512], f32) for i in range(2)]
    e_t = [sbuf(f"e{i}", [128, S], bf16) for i in range(NIT)]
    sume = [sbuf(f"sume{r}", [128, 1], f32) for r in range(4)]
    sumx = [sbuf(f"sumx{i}", [128, 1], f32) for i in range(NIT)]
    rr_ = [sbuf(f"rr{i}", [128, 1], f32) for i in range(NIT)]
    wb = [sbuf(f"wb{i}", [128, S], bf16) for i in range(NIT)]
    et = [sbuf(f"etg{g}", [128, 4, QBLK], bf16) for g in range(2)]
    oo = sbuf("oo", [H, QBLK], f32)

    orders = {
        "d": [("ksn", 16), ("qn", 16), ("idtcd", 16)],
        "e": [("vn", 16)],
        "b": [("bu", 16), ("bv", 16), ("bw", 16)],
        "c": [("cc", 1)],
        "g": [("gu", 16), ("gv", 16), ("gw", 16)],
        "o": [("owd0", 16), ("owd1", 16), ("ood", 16)],
        "p": [("qt", 1), ("kt", 1), ("dft", 1), ("mm0", 1), ("mm1", 1),
              ("mm2", 1), ("mm3", 1), ("tg0", 1), ("tg1", 1), ("av0", 1),
              ("av1", 1)],
        "a": [("p1", 1), ("p2", 1), ("sq0", 1), ("t0", 1), ("w0", 1),
              ("e0", 1), ("e1", 1), ("e2", 1), ("e3", 1), ("oo", 1)],
        "v": [("junk", 1), ("idtb", 1), ("cdb", 1), ("vb", 1), ("amp2_0", 1),
              ("uv0", 1), ("pa0", 1), ("pa1", 1), ("pa2", 1), ("pa3", 1),
              ("wb0", 1), ("wb1", 1), ("et0", 1), ("et1", 1)],
    }
    GRP = [[0, 1, 2, 3], [4, 5, 6, 7]]

    with nc.Block() as block, contextlib.ExitStack() as stack:
        sems = {s: stack.enter_context(nc.semaphore(f"s{s}"))
                for s in orders}
        L = Led(sems, orders)

        # ---------------- phase 1: input DMA, casts, transposes ----------
        with nc.psum_tensor("qt_ps", [128, QBLK], f32) as qt_h, \
             nc.psum_tensor("kt_ps", [128, QBLK], f32) as kt_h:
            qt_ps, kt_ps = qt_h.ap(), kt_h.ap()

            @block.sync
            def _(sp):
                L.inc(sp.dma_start(out=ksn[:], in_=Ks.rearrange(
                    "(t p) h -> p t h", p=128)), "d", "ksn")
                L.inc(sp.dma_start(out=qn[:], in_=Qs.rearrange(
                    "(t p) h -> p t h", p=128)), "d", "qn")
                L.inc(sp.dma_start(out=idtcd[:], in_=IDTCD[:]), "d", "idtcd")
                L.inc(sp.dma_start(out=vn[:], in_=Vd.rearrange(
                    "(t p) h -> p t h", p=128)), "e", "vn")

            @block.vector
            def _(dv):
                L.inc(dv.memset(junk[:], 1.0), "v", "junk")
                L.wait(dv, "idtcd")
                L.inc(dv.tensor_copy(idtb[:], idt[:]), "v", "idtb")
                L.inc(dv.tensor_copy(cdb[:], cdf[:]), "v", "cdb")
                L.wait(dv, "vn")
                L.inc(dv.tensor_copy(vb[:], vn[:]), "v", "vb")

            @block.tensor
            def _(pe):
                L.wait(pe, "idtcd")
                mi = None
                for t in range(NIT):
                    mi = pe.matmul(qt_ps[:, t * 128:(t + 1) * 128],
                                   qn[:, t, :], idt[:], is_transpose=True,
                                   start=True, stop=True)
                L.inc(mi, "p", "qt")
                for t in range(NIT):
                    mi = pe.matmul(kt_ps[:, t * 128:(t + 1) * 128],
                                   ksn[:, t, :], idt[:], is_transpose=True,
                                   start=True, stop=True)
                L.inc(mi, "p", "kt")

            @block.scalar
            def _(sc):
                L.wait(sc, "junk")
                sc.activation(junk2[:], junk[:], AF.Ln)  # ACT table preload
                L.wait(sc, "qt")
                L.inc(sc.copy(xtb[:, 0:QBLK], qt_ps[:]), "a", "p1")
                L.wait(sc, "kt")
                L.inc(sc.copy(xtb[:, QBLK:NFO], kt_ps[:]), "a", "p2")

        # ------------- phase 2: DFT + own features + allgather -----------
        with nc.psum_tensor("re_ps", [F, NFO], f32) as re_h, \
             nc.psum_tensor("im_ps", [F, NFO], f32) as im_h:
            re_ps, im_ps = re_h.ap(), im_h.ap()

            @block.tensor
            def _(pe):
                # re[0]/im[1] alias qt[0]/kt[1]; the p2 wait orders both
                L.wait(pe, "cdb")
                L.wait(pe, "p2")
                pe.matmul(re_ps[:], cdb[:, 0:F], xtb[:],
                          start=True, stop=True)
                L.inc(pe.matmul(im_ps[:], cdb[:, F:2 * F], xtb[:],
                                start=True, stop=True), "p", "dft")

            @block.scalar
            def _(sc):
                L.wait(sc, "dft")
                sc.activation(sqre[:], re_ps[:], AF.Square)
                L.inc(sc.activation(sqim[:], im_ps[:], AF.Square), "a", "sq0")
                L.wait(sc, "amp2_0")
                sc.activation(lg[:], amp2[:], AF.Ln)
                sc.drain()
                L.inc(sc.activation(tq[:], lg[:], AF.Exp, scale=-0.25),
                      "a", "t0")
                L.inc(sc.activation(wS[:], lg[:], AF.Exp, scale=0.25),
                      "a", "w0")

            @block.vector
            def _(dv):
                L.wait(dv, "sq0")
                L.inc(dv.tensor_add(amp2[:], sqre[:], sqim[:]), "v", "amp2_0")
                L.wait(dv, "t0")
                dv.tensor_mul(uS[:], re_ps[:], tq[:])
                L.inc(dv.tensor_mul(vS[:], im_ps[0:F - 1, :],
                                    tq[0:F - 1, :]), "v", "uv0")

            @block.sync
            def _(sp):
                L.wait(sp, "uv0")
                L.wait(sp, "w0")
                L.inc(sp.dma_start(out=b_all[0:F, :],
                                   in_=uS[:, QBLK:NFO]), "b", "bu")
                L.inc(sp.dma_start(out=b_all[F:2 * F - 1, :],
                                   in_=vS[:, QBLK:NFO]), "b", "bv")
                L.inc(sp.dma_start(out=b_all[2 * F - 1:NR, :],
                                   in_=wS[:, QBLK:NFO]), "b", "bw")

            @block.gpsimd
            def _(gp):
                L.wait(gp, "bw")
                L.inc(gp.collective_compute(
                    "AllGather", mybir.AluOpType.bypass,
                    replica_groups=GRP, ins=[b_all[:]], outs=[g_all[:]]),
                    "c", "cc")

            @block.sync
            def _(sp):
                L.wait(sp, "cc")
                L.inc(sp.dma_start(
                    out=uF[:], in_=g_all[:, 0:F, :].rearrange(
                        "c f s -> f c s")), "g", "gu")
                L.inc(sp.dma_start(
                    out=vF[:], in_=g_all[:, F:2 * F - 1, :].rearrange(
                        "c f s -> f c s")), "g", "gv")
                L.inc(sp.dma_start(
                    out=wF[:], in_=g_all[:, 2 * F - 1:NR, :].rearrange(
                        "c f s -> f c s")), "g", "gw")

        # ---------------- phase 3: scores, softmax, P@V -----------------
        with nc.psum_tensor("sc0", [128, S], f32) as s0h, \
             nc.psum_tensor("sc1", [128, S], f32) as s1h, \
             nc.psum_tensor("wt0", [128, 8 * 128], bf16) as w0h, \
             nc.psum_tensor("wt1", [128, 8 * 128], bf16) as w1h, \
             nc.psum_tensor("av_ps", [128, QBLK], f32) as avh:
            scp = [s0h.ap(), s1h.ap()]
            wt = [w0h.ap(), w1h.ap()]
            av_ps = avh.ap()
            rounds = [(0, 0), (1, 0), (0, 1), (1, 1)]  # (it, hf); slot=r%2

            def kf(t, hf):  # gathered K-feature rhs slice, 512 cols
                return t[:, 2 * hf:2 * hf + 2, :]

            @block.tensor
            def _(pe):
                for r, (it, hf) in enumerate(rounds):
                    i0, i1 = it * 128, (it + 1) * 128
                    slot = scp[r % 2]
                    if r == 0:
                        L.wait(pe, "gw")   # gathered features (sc0 aliases
                    elif r >= 2:           # re/im; gw follows uv0/sq0)
                        L.wait(pe, f"pa{r - 2}")  # sc slot reuse
                    pe.matmul(slot[:, 0:512], uS[:, i0:i1], kf(uF, hf),
                              start=True, stop=False)
                    pe.matmul(slot[:, 0:512], vS[:, i0:i1], kf(vF, hf),
                              start=False, stop=True)
                    L.inc(pe.matmul(slot[:, 512:1024], wS[:, i0:i1],
                                    kf(wF, hf), start=True, stop=True),
                          "p", f"mm{r}")
                for g in range(2):
                    mi = None
                    for it in range(NIT):
                        L.wait(pe, f"wb{it}")
                        for lj in range(4):
                            jt = g * 4 + lj
                            mi = pe.matmul(
                                wt[g][:, lj * 2 * 128 + it * 128:
                                      lj * 2 * 128 + (it + 1) * 128],
                                wb[it][:, jt * 128:(jt + 1) * 128], idtb[:],
                                is_transpose=True, start=True, stop=True)
                    L.inc(mi, "p", f"tg{g}")
                for g in range(2):
                    L.wait(pe, f"et{g}")
                    mi = None
                    for lj in range(4):
                        jt = g * 4 + lj
                        mi = pe.matmul(av_ps[:], vb[:, jt, :],
                                       et[g][:, lj, :],
                                       start=(jt == 0), stop=(jt == NJT - 1))
                    L.inc(mi, "p", f"av{g}")

            @block.vector
            def _(dv):
                for r, (it, hf) in enumerate(rounds):
                    L.wait(dv, f"mm{r}")
                    if r >= 2:
                        L.wait(dv, f"e{r - 2}")
                    slot = scp[r % 2]
                    dv.drain()
                    dv.reciprocal_approx_fast(out=inv[:],
                                              in_=slot[:, 512:1024])
                    dv.drain()
                    L.inc(dv.tensor_mul(pa[r % 2][:], slot[:, 0:512],
                                        inv[:]), "v", f"pa{r}")
                for it in range(NIT):
                    L.wait(dv, f"e{it + 2}")
                    dv.tensor_add(sumx[it][:], sume[it][:], sume[it + 2][:])
                    dv.drain()
                    dv.reciprocal(rr_[it][:], sumx[it][:])
                    dv.drain()
                    L.inc(dv.tensor_scalar_mul(wb[it][:], e_t[it][:],
                                               rr_[it][:]), "v", f"wb{it}")
                for g in range(2):
                    L.wait(dv, f"tg{g}")
                    L.inc(dv.tensor_copy(et[g][:], wt[g][:]), "v", f"et{g}")

            @block.scalar
            def _(sc):
                for r, (it, hf) in enumerate(rounds):
                    L.wait(sc, f"pa{r}")
                    L.inc(sc.activation(e_t[it][:, hf * 512:(hf + 1) * 512],
                                        pa[r % 2][:], AF.Exp, bias=1.0,
                                        accum_out=sume[r][:]), "a", f"e{r}")
                L.wait(sc, "av1")
                L.inc(sc.copy(oo[:], av_ps[:]), "a", "oo")

            @block.sync
            def _(sp):
                for it in range(NIT):
                    L.wait(sp, f"wb{it}")
                    L.inc(sp.dma_start(out=OW[it * 128:(it + 1) * 128, :],
                                       in_=wb[it][:]), "o", f"owd{it}")
                L.wait(sp, "oo")
                L.inc(sp.dma_start(out=OO[:], in_=oo[:]), "o", "ood")
                L.wait(sp, "ood")

    nc.compile()
    return nc


def _get_nc():
    if "nc" not in _CACHE:
        _CACHE["nc"] = _build()
    return _CACHE["nc"]


def kernel(Q, K, V):
    from concourse.bass_utils import run_bass_kernel_spmd

    Q = np.ascontiguousarray(np.asarray(Q, dtype=np.float32))
    K = np.ascontiguousarray(np.asarray(K, dtype=np.float32))
    V = np.ascontiguousarray(np.asarray(V, dtype=np.float32))
    nc = _get_nc()
    in_maps = []
    for c in range(NCORES):
        b, qb = c // 4, c % 4
        in_maps.append({
            "Qs": np.ascontiguousarray(Q[b, qb * QBLK:(qb + 1) * QBLK]),
            "Ks": np.ascontiguousarray(K[b, qb * QBLK:(qb + 1) * QBLK]),
            "V": V[b],
        })
    res = run_bass_kernel_spmd(nc, in_maps, core_ids=list(range(NCORES)))
    output = np.empty((B, S, H), np.float32)
    weights = np.empty((B, S, S), np.float32)
    for c in range(NCORES):
        b, qb = c // 4, c % 4
        rr = res.results[c]
        weights[b, qb * QBLK:(qb + 1) * QBLK, :] = rr["OW"].astype(np.float32)
        output[b, qb * QBLK:(qb + 1) * QBLK, :] = rr["OO"].T
    return output, weights


if __name__ == "__main__":
    rng = np.random.default_rng(0)
    Q = rng.standard_normal((B, S, H)).astype(np.float32)
    K = rng.standard_normal((B, S, H)).astype(np.float32)
    V = rng.standard_normal((B, S, H)).astype(np.float32)
    out, w = kernel(Q, K, V)
    print("kernel ran:", out.shape, w.shape)


# revision 20
# speedup vs baseline: 2.2305x; 2.2305x over previous
"""AmplitudeWeightedPhaseAttention Trainium2 kernel (8 NeuronCores, SPMD).

Math: the reference's [B,Sq,Sk,F] tensor collapses algebraically.
With rfft bin features re/im and amp2 = re^2 + im^2:
    t  = amp2^(-1/4)       u = re*t   v = im*t   w = amp2^(+1/4)
    num[i,j] = sum_f u_q u_k + v_q v_k        (v==0 at f=0 and f=64)
    den[i,j] = sum_f w_q w_k                  (rank-65 matmul)
    weights  = softmax_j(num/den + 1)         out = weights @ V
Sharding: core c owns batch c//4, query rows (c%4)*256..+256.  Each core
computes its [256, 1024] score block in natural [i_p, j] layout (softmax
normalization is then a per-partition scalar), transposes the normalized
weights via PE to feed the P@V matmul, and writes its weights block and
(transposed) output block.  No collectives.
"""

import numpy as np
from contextlib import ExitStack

B, S, H = 2, 1024, 128
F = H // 2 + 1  # 65
NCORES = 8
QBLK = S // 4  # 256 query rows per core
NJT = S // 128  # 8 key tiles
NIT = QBLK // 128  # 2 query sub-tiles

WEIGHTS_BF16 = True  # store weights output as bf16, upcast on host

_CACHE = {}


def _dft_consts():
    h = np.arange(H, dtype=np.float64)[:, None]
    f = np.arange(F, dtype=np.float64)[None, :]
    C = np.cos(2 * np.pi * h * f / H)
    Sn = -np.sin(2 * np.pi * h * f / H)
    Sn[:, 0] = 0.0
    Sn[:, F - 1] = 0.0  # exactly zero at DC and Nyquist
    return np.concatenate([C, Sn], axis=1).astype(np.float32)  # [128, 130]


def _patch_act_tables():
    """Make Ln and Exp both first-match to natural_log_exp_and_others so the
    whole kernel needs a single ACT table load. Set IDs stay canonical (we
    only edit membership of the shadowing sets, keyed by name)."""
    import concourse.bacc as bacc_mod
    from concourse import mybir
    if getattr(bacc_mod, "_awpa_tables_patched", False):
        return
    orig = bacc_mod.get_activation_tables
    AF = mybir.ActivationFunctionType

    def patched(arch):
        tables = dict(orig(arch))
        if "natural_log_exp_and_others" in tables:
            for name, fns in tables.items():
                if name != "natural_log_exp_and_others" and \
                        (AF.Ln in fns or AF.Exp in fns):
                    tables[name] = fns - {AF.Ln, AF.Exp}
        return tables

    bacc_mod.get_activation_tables = patched
    bacc_mod._awpa_tables_patched = True


def _build():
    import concourse.bass as bass
    import concourse.tile as tile
    from concourse import bacc, mybir

    _patch_act_tables()
    f32 = mybir.dt.float32
    bf16 = mybir.dt.bfloat16
    AF = mybir.ActivationFunctionType

    nc = bacc.Bacc("TRN2", target_bir_lowering=False, debug=False,
                   num_devices=NCORES)
    Qs = nc.dram_tensor("Qs", [QBLK, H], f32, kind="ExternalInput").ap()
    K = nc.dram_tensor("K", [S, H], f32, kind="ExternalInput").ap()
    V = nc.dram_tensor("V", [S, H], f32, kind="ExternalInput").ap()
    CD = nc.inline_tensor(_dft_consts(), "CDconst").ap()  # [128, 130]
    IDT = nc.inline_tensor(np.eye(H, dtype=np.float32), "IDTconst").ap()
    ow_dt = bf16 if WEIGHTS_BF16 else f32
    OW = nc.dram_tensor("OW", [QBLK, S], ow_dt, kind="ExternalOutput").ap()
    OO = nc.dram_tensor("OO", [H, QBLK], f32, kind="ExternalOutput").ap()

    with ExitStack() as ctx:
        tc = ctx.enter_context(tile.TileContext(nc))
        consts = ctx.enter_context(tc.tile_pool(name="consts", bufs=1))
        big = ctx.enter_context(tc.tile_pool(name="big", bufs=1))
        ftmp = ctx.enter_context(tc.tile_pool(name="ftmp", bufs=1))
        ps = ctx.enter_context(tc.tile_pool(name="ps", bufs=4, space="PSUM"))

        # ACT table preload: first ACT op is Ln; with the patched tables one
        # load of natural_log_exp_and_others covers Ln/Exp/Square/Copy.
        junk = consts.tile([128, 1], f32)
        nc.vector.memset(junk[:], 1.0)
        junk2 = consts.tile([128, 1], f32)
        nc.scalar.activation(junk2[:], junk[:], AF.Ln)

        # --- DMA inputs ---
        kn = big.tile([128, NJT, H], f32)
        nc.sync.dma_start(out=kn[:], in_=K.rearrange("(t p) h -> p t h", p=128))
        idt = consts.tile([H, H], f32)
        nc.sync.dma_start(out=idt[:], in_=IDT[:])
        qn = big.tile([128, NIT, H], f32)
        nc.sync.dma_start(out=qn[:], in_=Qs.rearrange("(t p) h -> p t h", p=128))
        cdf = consts.tile([H, 2 * F], f32)
        nc.sync.dma_start(out=cdf[:], in_=CD[:])
        vn = big.tile([128, NJT, H], f32)
        nc.sync.dma_start(out=vn[:], in_=V.rearrange("(t p) h -> p t h", p=128))
        cdb = consts.tile([H, 2 * F], bf16)
        nc.vector.tensor_copy(cdb[:], cdf[:])
        vb = big.tile([128, NJT, H], bf16)
        nc.vector.tensor_copy(vb[:], vn[:])

        # --- transposes: K^T, Q^T (h on partitions) via PE ---
        kt_ps = ps.tile([128, S], f32, tag="ps")
        for t in range(NJT):
            nc.tensor.transpose(kt_ps[:, t * 128:(t + 1) * 128], kn[:, t, :],
                                idt[:])
        ktb = big.tile([128, S], bf16)
        nc.scalar.copy(ktb[:], kt_ps[:])

        qt_ps = ps.tile([128, QBLK], f32, tag="ps")
        for t in range(NIT):
            nc.tensor.transpose(qt_ps[:, t * 128:(t + 1) * 128], qn[:, t, :],
                                idt[:])
        qtb = big.tile([128, QBLK], bf16)
        nc.vector.tensor_copy(qtb[:], qt_ps[:])

        # --- DFT + features for X in {K (N=1024), Q (N=256)} ---
        def dft_ln_phase(xtb, N, qk):
            re_ps = ps.tile([F, N], f32, tag="ps")
            im_ps = ps.tile([F, N], f32, tag="ps")
            for c0 in range(0, N, 512):
                c1 = min(c0 + 512, N)
                nc.tensor.matmul(re_ps[:, c0:c1], cdb[:, 0:F], xtb[:, c0:c1],
                                 start=True, stop=True)
                nc.tensor.matmul(im_ps[:, c0:c1], cdb[:, F:2 * F],
                                 xtb[:, c0:c1], start=True, stop=True)
            sqre = ftmp.tile([F, N], bf16, tag=f"sqre{qk}")
            nc.scalar.activation(sqre[:], re_ps[:], AF.Square)
            sqim = ftmp.tile([F, N], bf16, tag=f"sqim{qk}")
            nc.scalar.activation(sqim[:], im_ps[:], AF.Square)
            amp2 = ftmp.tile([F, N], bf16, tag=f"amp2{qk}")
            nc.vector.tensor_add(amp2[:], sqre[:], sqim[:])
            lg = ftmp.tile([F, N], f32, tag=f"lg{qk}")
            nc.scalar.activation(lg[:], amp2[:], AF.Ln)
            return re_ps, im_ps, lg

        def exp_phase(re_ps, im_ps, lg, N, Ux, Vx, Wx, qk):
            tq = ftmp.tile([F, N], f32, tag=f"tq{qk}")
            nc.scalar.activation(tq[:], lg[:], AF.Exp, scale=-0.25)
            nc.scalar.activation(Wx[:], lg[:], AF.Exp, scale=0.25)
            nc.vector.tensor_mul(Ux[:], re_ps[:], tq[:])
            # v(f)=im(f)*t(f) for f=0..63; v(0)==0 since im(0)==0
            nc.vector.tensor_mul(Vx[:], im_ps[0:F - 1, :], tq[0:F - 1, :])

        uk = big.tile([F, S], bf16)
        vk = big.tile([F - 1, S], bf16)
        wk = big.tile([F, S], bf16)
        uq = big.tile([F, QBLK], bf16)
        vq = big.tile([F - 1, QBLK], bf16)
        wq = big.tile([F, QBLK], bf16)
        rek, imk, lgk = dft_ln_phase(ktb, S, "k")
        req, imq, lgq = dft_ln_phase(qtb, QBLK, "q")
        exp_phase(rek, imk, lgk, S, uk, vk, wk, "k")
        exp_phase(req, imq, lgq, QBLK, uq, vq, wq, "q")

        # --- scores + softmax per query sub-tile (natural [i_p, j]) ---
        wb_tiles = []
        for it in range(NIT):
            i0, i1 = it * 128, (it + 1) * 128
            num_ps = ps.tile([128, S], f32, tag="ps")
            den_ps = ps.tile([128, S], f32, tag="ps")
            for c0 in range(0, S, 512):
                c1 = c0 + 512
                nc.tensor.matmul(num_ps[:, c0:c1], uq[:, i0:i1], uk[:, c0:c1],
                                 start=True, stop=False)
                nc.tensor.matmul(num_ps[:, c0:c1], vq[:, i0:i1], vk[:, c0:c1],
                                 start=False, stop=True)
                nc.tensor.matmul(den_ps[:, c0:c1], wq[:, i0:i1], wk[:, c0:c1],
                                 start=True, stop=True)
            inv = ftmp.tile([128, S], f32, tag="inv")
            nc.vector.reciprocal_approx_fast(out=inv[:], in_=den_ps[:])
            pa = ftmp.tile([128, S], f32, tag="pa")
            nc.vector.tensor_mul(pa[:], num_ps[:], inv[:])
            e = ftmp.tile([128, S], bf16, tag="e")
            sumexp = ftmp.tile([128, 1], f32, tag="sumexp")
            nc.scalar.activation(e[:], pa[:], AF.Exp, bias=1.0,
                                 accum_out=sumexp[:])
            r = ftmp.tile([128, 1], f32, tag="r")
            nc.vector.reciprocal(r[:], sumexp[:])
            wb = big.tile([128, S], bf16, tag=f"wb{it}")
            nc.vector.tensor_scalar_mul(wb[:], e[:], r[:])
            wb_tiles.append(wb)
            if WEIGHTS_BF16:
                nc.sync.dma_start(out=OW[i0:i1, :], in_=wb[:])
            else:
                oww = ftmp.tile([128, S], f32, tag="oww")
                nc.vector.tensor_scalar_mul(oww[:], e[:], r[:])
                nc.sync.dma_start(out=OW[i0:i1, :], in_=oww[:])

        # --- transpose normalized weights to [j_p, i] for P@V ---
        idtb = consts.tile([H, H], bf16)
        nc.vector.tensor_copy(idtb[:], idt[:])
        et_tiles = []
        for g in range(2):  # two groups of 4 j-tiles share one psum tile
            wt_ps = ps.tile([128, 4 * QBLK], bf16, tag="ps")
            for lj in range(4):
                jt = g * 4 + lj
                for it in range(NIT):
                    nc.tensor.transpose(
                        wt_ps[:, lj * QBLK + it * 128:
                              lj * QBLK + (it + 1) * 128],
                        wb_tiles[it][:, jt * 128:(jt + 1) * 128], idtb[:])
            for lj in range(4):
                jt = g * 4 + lj
                et = big.tile([128, QBLK], bf16, tag=f"et{jt}")
                nc.scalar.copy(et[:], wt_ps[:, lj * QBLK:(lj + 1) * QBLK])
                et_tiles.append(et)

        # --- P@V: out^T[h, i] accumulated over j tiles ---
        av_ps = ps.tile([128, QBLK], f32, tag="ps")
        for jt in range(NJT):
            nc.tensor.matmul(av_ps[:], vb[:, jt, :], et_tiles[jt][:],
                             start=(jt == 0), stop=(jt == NJT - 1))
        oo = big.tile([H, QBLK], f32)
        nc.vector.tensor_copy(oo[:], av_ps[:])
        nc.sync.dma_start(out=OO[:], in_=oo[:])

    nc.compile()
    return nc


def _get_nc():
    if "nc" not in _CACHE:
        _CACHE["nc"] = _build()
    return _CACHE["nc"]


def kernel(Q, K, V):
    from concourse.bass_utils import run_bass_kernel_spmd

    Q = np.ascontiguousarray(np.asarray(Q, dtype=np.float32))
    K = np.ascontiguousarray(np.asarray(K, dtype=np.float32))
    V = np.ascontiguousarray(np.asarray(V, dtype=np.float32))
    nc = _get_nc()
    in_maps = []
    for c in range(NCORES):
        b, qb = c // 4, c % 4
        in_maps.append({
            "Qs": np.ascontiguousarray(Q[b, qb * QBLK:(qb + 1) * QBLK]),
            "K": K[b],
            "V": V[b],
        })
    res = run_bass_kernel_spmd(nc, in_maps, core_ids=list(range(NCORES)))
    output = np.empty((B, S, H), np.float32)
    weights = np.empty((B, S, S), np.float32)
    for c in range(NCORES):
        b, qb = c // 4, c % 4
        rr = res.results[c]
        weights[b, qb * QBLK:(qb + 1) * QBLK, :] = rr["OW"].astype(np.float32)
        output[b, qb * QBLK:(qb + 1) * QBLK, :] = rr["OO"].T
    return output, weights


if __name__ == "__main__":
    rng = np.random.default_rng(0)
    Q = rng.standard_normal((B, S, H)).astype(np.float32)
    K = rng.standard_normal((B, S, H)).astype(np.float32)
    V = rng.standard_normal((B, S, H)).astype(np.float32)
    out, w = kernel(Q, K, V)
    print("kernel ran:", out.shape, w.shape)


# revision 21
# speedup vs baseline: 2.2590x; 1.0128x over previous
"""AmplitudeWeightedPhaseAttention Trainium2 kernel (8 NeuronCores, SPMD).

Math: the reference's [B,Sq,Sk,F] tensor collapses algebraically.
With rfft bin features re/im and amp2 = re^2 + im^2:
    t  = amp2^(-1/4)       u = re*t   v = im*t   w = amp2^(+1/4)
    num[i,j] = sum_f u_q u_k + v_q v_k        (v==0 at f=0 and f=64)
    den[i,j] = sum_f w_q w_k                  (rank-65 matmul)
    weights  = softmax_j(num/den + 1)         out = weights @ V
Sharding: core c owns batch c//4, query rows (c%4)*256..+256.  Each core
computes its [256, 1024] score block in natural [i_p, j] layout (softmax
normalization is then a per-partition scalar), transposes the normalized
weights via PE to feed the P@V matmul, and writes its weights block and
(transposed) output block.  No collectives.
"""

import numpy as np
from contextlib import ExitStack

B, S, H = 2, 1024, 128
F = H // 2 + 1  # 65
NCORES = 8
QBLK = S // 4  # 256 query rows per core
NJT = S // 128  # 8 key tiles
NIT = QBLK // 128  # 2 query sub-tiles

WEIGHTS_BF16 = True  # store weights output as bf16, upcast on host

_CACHE = {}


def _dft_consts():
    h = np.arange(H, dtype=np.float64)[:, None]
    f = np.arange(F, dtype=np.float64)[None, :]
    C = np.cos(2 * np.pi * h * f / H)
    Sn = -np.sin(2 * np.pi * h * f / H)
    Sn[:, 0] = 0.0
    Sn[:, F - 1] = 0.0  # exactly zero at DC and Nyquist
    return np.concatenate([C, Sn], axis=1).astype(np.float32)  # [128, 130]


def _patch_act_tables():
    """Make Ln and Exp both first-match to natural_log_exp_and_others so the
    whole kernel needs a single ACT table load. Set IDs stay canonical (we
    only edit membership of the shadowing sets, keyed by name)."""
    import concourse.bacc as bacc_mod
    from concourse import mybir
    if getattr(bacc_mod, "_awpa_tables_patched", False):
        return
    orig = bacc_mod.get_activation_tables
    AF = mybir.ActivationFunctionType

    def patched(arch):
        tables = dict(orig(arch))
        if "natural_log_exp_and_others" in tables:
            for name, fns in tables.items():
                if name != "natural_log_exp_and_others" and \
                        (AF.Ln in fns or AF.Exp in fns):
                    tables[name] = fns - {AF.Ln, AF.Exp}
        return tables

    bacc_mod.get_activation_tables = patched
    bacc_mod._awpa_tables_patched = True


def _build():
    import concourse.bass as bass
    import concourse.tile as tile
    from concourse import bacc, mybir

    _patch_act_tables()
    f32 = mybir.dt.float32
    bf16 = mybir.dt.bfloat16
    AF = mybir.ActivationFunctionType

    nc = bacc.Bacc("TRN2", target_bir_lowering=False, debug=False,
                   num_devices=NCORES)
    Qs = nc.dram_tensor("Qs", [QBLK, H], f32, kind="ExternalInput").ap()
    K = nc.dram_tensor("K", [S, H], f32, kind="ExternalInput").ap()
    V = nc.dram_tensor("V", [S, H], f32, kind="ExternalInput").ap()
    CD = nc.inline_tensor(_dft_consts(), "CDconst").ap()  # [128, 130]
    IDT = nc.inline_tensor(np.eye(H, dtype=np.float32), "IDTconst").ap()
    ow_dt = bf16 if WEIGHTS_BF16 else f32
    OW = nc.dram_tensor("OW", [QBLK, S], ow_dt, kind="ExternalOutput").ap()
    OO = nc.dram_tensor("OO", [H, QBLK], f32, kind="ExternalOutput").ap()

    with ExitStack() as ctx:
        tc = ctx.enter_context(tile.TileContext(nc))
        consts = ctx.enter_context(tc.tile_pool(name="consts", bufs=1))
        big = ctx.enter_context(tc.tile_pool(name="big", bufs=1))
        ftmp = ctx.enter_context(tc.tile_pool(name="ftmp", bufs=1))
        ps = ctx.enter_context(tc.tile_pool(name="ps", bufs=4, space="PSUM"))

        # ACT table preload: first ACT op is Ln; with the patched tables one
        # load of natural_log_exp_and_others covers Ln/Exp/Square/Copy.
        junk = consts.tile([128, 1], f32)
        nc.vector.memset(junk[:], 1.0)
        junk2 = consts.tile([128, 1], f32)
        nc.scalar.activation(junk2[:], junk[:], AF.Ln)

        # --- DMA inputs ---
        kn = big.tile([128, NJT, H], f32)
        nc.sync.dma_start(out=kn[:], in_=K.rearrange("(t p) h -> p t h", p=128))
        idt = consts.tile([H, H], f32)
        nc.sync.dma_start(out=idt[:], in_=IDT[:])
        qn = big.tile([128, NIT, H], f32)
        nc.sync.dma_start(out=qn[:], in_=Qs.rearrange("(t p) h -> p t h", p=128))
        cdf = consts.tile([H, 2 * F], f32)
        nc.sync.dma_start(out=cdf[:], in_=CD[:])
        vn = big.tile([128, NJT, H], f32)
        nc.sync.dma_start(out=vn[:], in_=V.rearrange("(t p) h -> p t h", p=128))
        cdb = consts.tile([H, 2 * F], bf16)
        nc.vector.tensor_copy(cdb[:], cdf[:])
        vb = big.tile([128, NJT, H], bf16)
        nc.gpsimd.tensor_copy(vb[:], vn[:])

        # --- transposes: K^T, Q^T (h on partitions) via PE ---
        kt_ps = ps.tile([128, S], f32, tag="ps")
        for t in range(NJT):
            nc.tensor.transpose(kt_ps[:, t * 128:(t + 1) * 128], kn[:, t, :],
                                idt[:])
        ktb = big.tile([128, S], bf16)
        nc.scalar.copy(ktb[:], kt_ps[:])

        qt_ps = ps.tile([128, QBLK], f32, tag="ps")
        for t in range(NIT):
            nc.tensor.transpose(qt_ps[:, t * 128:(t + 1) * 128], qn[:, t, :],
                                idt[:])
        qtb = big.tile([128, QBLK], bf16)
        nc.vector.tensor_copy(qtb[:], qt_ps[:])

        # --- DFT + features for X in {K (N=1024), Q (N=256)} ---
        def dft_ln_phase(xtb, N, qk):
            re_ps = ps.tile([F, N], f32, tag="ps")
            im_ps = ps.tile([F, N], f32, tag="ps")
            for c0 in range(0, N, 512):
                c1 = min(c0 + 512, N)
                nc.tensor.matmul(re_ps[:, c0:c1], cdb[:, 0:F], xtb[:, c0:c1],
                                 start=True, stop=True)
                nc.tensor.matmul(im_ps[:, c0:c1], cdb[:, F:2 * F],
                                 xtb[:, c0:c1], start=True, stop=True)
            sqre = ftmp.tile([F, N], bf16, tag=f"sqre{qk}")
            nc.scalar.activation(sqre[:], re_ps[:], AF.Square)
            sqim = ftmp.tile([F, N], bf16, tag=f"sqim{qk}")
            nc.scalar.activation(sqim[:], im_ps[:], AF.Square)
            amp2 = ftmp.tile([F, N], bf16, tag=f"amp2{qk}")
            nc.vector.tensor_add(amp2[:], sqre[:], sqim[:])
            lg = ftmp.tile([F, N], f32, tag=f"lg{qk}")
            nc.scalar.activation(lg[:], amp2[:], AF.Ln)
            return re_ps, im_ps, lg

        def exp_phase(re_ps, im_ps, lg, N, Ux, Vx, Wx, qk):
            tq = ftmp.tile([F, N], f32, tag=f"tq{qk}")
            nc.scalar.activation(tq[:], lg[:], AF.Exp, scale=-0.25)
            nc.scalar.activation(Wx[:], lg[:], AF.Exp, scale=0.25)
            nc.vector.tensor_mul(Ux[:], re_ps[:], tq[:])
            # v(f)=im(f)*t(f) for f=0..63; v(0)==0 since im(0)==0
            nc.vector.tensor_mul(Vx[:], im_ps[0:F - 1, :], tq[0:F - 1, :])

        uk = big.tile([F, S], bf16)
        vk = big.tile([F - 1, S], bf16)
        wk = big.tile([F, S], bf16)
        uq = big.tile([F, QBLK], bf16)
        vq = big.tile([F - 1, QBLK], bf16)
        wq = big.tile([F, QBLK], bf16)
        rek, imk, lgk = dft_ln_phase(ktb, S, "k")
        req, imq, lgq = dft_ln_phase(qtb, QBLK, "q")
        exp_phase(rek, imk, lgk, S, uk, vk, wk, "k")
        exp_phase(req, imq, lgq, QBLK, uq, vq, wq, "q")

        # --- scores + softmax per query sub-tile (natural [i_p, j]) ---
        wb_tiles = []
        for it in range(NIT):
            i0, i1 = it * 128, (it + 1) * 128
            num_ps = ps.tile([128, S], f32, tag="ps")
            den_ps = ps.tile([128, S], f32, tag="ps")
            for c0 in range(0, S, 512):
                c1 = c0 + 512
                nc.tensor.matmul(num_ps[:, c0:c1], uq[:, i0:i1], uk[:, c0:c1],
                                 start=True, stop=False)
                nc.tensor.matmul(num_ps[:, c0:c1], vq[:, i0:i1], vk[:, c0:c1],
                                 start=False, stop=True)
                nc.tensor.matmul(den_ps[:, c0:c1], wq[:, i0:i1], wk[:, c0:c1],
                                 start=True, stop=True)
            inv = ftmp.tile([128, S], f32, tag="inv")
            nc.vector.reciprocal_approx_fast(out=inv[:], in_=den_ps[:])
            pa = ftmp.tile([128, S], f32, tag="pa")
            nc.vector.tensor_mul(pa[:], num_ps[:], inv[:])
            e = ftmp.tile([128, S], bf16, tag="e")
            sumexp = ftmp.tile([128, 1], f32, tag="sumexp")
            nc.scalar.activation(e[:], pa[:], AF.Exp, bias=1.0,
                                 accum_out=sumexp[:])
            r = ftmp.tile([128, 1], f32, tag="r")
            nc.vector.reciprocal(r[:], sumexp[:])
            wb = big.tile([128, S], bf16, tag=f"wb{it}")
            nc.vector.tensor_scalar_mul(wb[:], e[:], r[:])
            wb_tiles.append(wb)
            if WEIGHTS_BF16:
                nc.sync.dma_start(out=OW[i0:i1, :], in_=wb[:])
            else:
                oww = ftmp.tile([128, S], f32, tag="oww")
                nc.vector.tensor_scalar_mul(oww[:], e[:], r[:])
                nc.sync.dma_start(out=OW[i0:i1, :], in_=oww[:])

        # --- transpose normalized weights to [j_p, i] for P@V ---
        idtb = consts.tile([H, H], bf16)
        nc.vector.tensor_copy(idtb[:], idt[:])
        et_tiles = []
        for g in range(2):  # two groups of 4 j-tiles share one psum tile
            wt_ps = ps.tile([128, 4 * QBLK], bf16, tag="ps")
            for lj in range(4):
                jt = g * 4 + lj
                for it in range(NIT):
                    nc.tensor.transpose(
                        wt_ps[:, lj * QBLK + it * 128:
                              lj * QBLK + (it + 1) * 128],
                        wb_tiles[it][:, jt * 128:(jt + 1) * 128], idtb[:])
            for lj in range(4):
                jt = g * 4 + lj
                et = big.tile([128, QBLK], bf16, tag=f"et{jt}")
                eng = nc.vector.tensor_copy if (jt % 2) else nc.scalar.copy
                eng(et[:], wt_ps[:, lj * QBLK:(lj + 1) * QBLK])
                et_tiles.append(et)

        # --- P@V: out^T[h, i] accumulated over j tiles ---
        av_ps = ps.tile([128, QBLK], f32, tag="ps")
        for jt in range(NJT):
            nc.tensor.matmul(av_ps[:], vb[:, jt, :], et_tiles[jt][:],
                             start=(jt == 0), stop=(jt == NJT - 1))
        oo = big.tile([H, QBLK], f32)
        nc.vector.tensor_copy(oo[:], av_ps[:])
        nc.sync.dma_start(out=OO[:], in_=oo[:])

    nc.compile()
    return nc


def _get_nc():
    if "nc" not in _CACHE:
        _CACHE["nc"] = _build()
    return _CACHE["nc"]


def kernel(Q, K, V):
    from concourse.bass_utils import run_bass_kernel_spmd

    Q = np.ascontiguousarray(np.asarray(Q, dtype=np.float32))
    K = np.ascontiguousarray(np.asarray(K, dtype=np.float32))
    V = np.ascontiguousarray(np.asarray(V, dtype=np.float32))
    nc = _get_nc()
    in_maps = []
    for c in range(NCORES):
        b, qb = c // 4, c % 4
        in_maps.append({
            "Qs": np.ascontiguousarray(Q[b, qb * QBLK:(qb + 1) * QBLK]),
            "K": K[b],
            "V": V[b],
        })
    res = run_bass_kernel_spmd(nc, in_maps, core_ids=list(range(NCORES)))
    output = np.empty((B, S, H), np.float32)
    weights = np.empty((B, S, S), np.float32)
    for c in range(NCORES):
        b, qb = c // 4, c % 4
        rr = res.results[c]
        weights[b, qb * QBLK:(qb + 1) * QBLK, :] = rr["OW"].astype(np.float32)
        output[b, qb * QBLK:(qb + 1) * QBLK, :] = rr["OO"].T
    return output, weights


if __name__ == "__main__":
    rng = np.random.default_rng(0)
    Q = rng.standard_normal((B, S, H)).astype(np.float32)
    K = rng.standard_normal((B, S, H)).astype(np.float32)
    V = rng.standard_normal((B, S, H)).astype(np.float32)
    out, w = kernel(Q, K, V)
    print("kernel ran:", out.shape, w.shape)


# revision 23
# speedup vs baseline: 2.3593x; 1.0444x over previous
"""AmplitudeWeightedPhaseAttention Trainium2 kernel (8 NeuronCores, SPMD).

Math: the reference's [B,Sq,Sk,F] tensor collapses algebraically.
With rfft bin features re/im and amp2 = re^2 + im^2:
    t  = amp2^(-1/4)       u = re*t   v = im*t   w = amp2^(+1/4)
    num[i,j] = sum_f u_q u_k + v_q v_k        (v==0 at f=0 and f=64)
    den[i,j] = sum_f w_q w_k                  (rank-65 matmul)
    weights  = softmax_j(num/den + 1)         out = weights @ V
Sharding: core c owns batch c//4, query rows (c%4)*256..+256.  Each core
computes its [256, 1024] score block in natural [i_p, j] layout (softmax
normalization is then a per-partition scalar), transposes the normalized
weights via PE to feed the P@V matmul, and writes its weights block and
(transposed) output block.  No collectives.
"""

import numpy as np
from contextlib import ExitStack

B, S, H = 2, 1024, 128
F = H // 2 + 1  # 65
NCORES = 8
QBLK = S // 4  # 256 query rows per core
NJT = S // 128  # 8 key tiles
NIT = QBLK // 128  # 2 query sub-tiles

WEIGHTS_BF16 = True  # store weights output as bf16, upcast on host

_CACHE = {}


def _dft_consts():
    h = np.arange(H, dtype=np.float64)[:, None]
    f = np.arange(F, dtype=np.float64)[None, :]
    C = np.cos(2 * np.pi * h * f / H)
    Sn = -np.sin(2 * np.pi * h * f / H)
    Sn[:, 0] = 0.0
    Sn[:, F - 1] = 0.0  # exactly zero at DC and Nyquist
    return np.concatenate([C, Sn], axis=1).astype(np.float32)  # [128, 130]


def _patch_act_tables():
    """Make Ln and Exp both first-match to natural_log_exp_and_others so the
    whole kernel needs a single ACT table load. Set IDs stay canonical (we
    only edit membership of the shadowing sets, keyed by name)."""
    import concourse.bacc as bacc_mod
    from concourse import mybir
    if getattr(bacc_mod, "_awpa_tables_patched", False):
        return
    orig = bacc_mod.get_activation_tables
    AF = mybir.ActivationFunctionType

    def patched(arch):
        tables = dict(orig(arch))
        if "natural_log_exp_and_others" in tables:
            for name, fns in tables.items():
                if name != "natural_log_exp_and_others" and \
                        (AF.Ln in fns or AF.Exp in fns):
                    tables[name] = fns - {AF.Ln, AF.Exp}
        return tables

    bacc_mod.get_activation_tables = patched
    bacc_mod._awpa_tables_patched = True


def _build():
    import concourse.bass as bass
    import concourse.tile as tile
    from concourse import bacc, mybir

    _patch_act_tables()
    f32 = mybir.dt.float32
    bf16 = mybir.dt.bfloat16
    AF = mybir.ActivationFunctionType

    nc = bacc.Bacc("TRN2", target_bir_lowering=False, debug=False,
                   num_devices=NCORES)
    Qs = nc.dram_tensor("Qs", [QBLK, H], f32, kind="ExternalInput").ap()
    K = nc.dram_tensor("K", [S, H], f32, kind="ExternalInput").ap()
    V = nc.dram_tensor("V", [S, H], f32, kind="ExternalInput").ap()
    CD = nc.inline_tensor(_dft_consts(), "CDconst").ap()  # [128, 130]
    IDT = nc.inline_tensor(np.eye(H, dtype=np.float32), "IDTconst").ap()
    ow_dt = bf16 if WEIGHTS_BF16 else f32
    OW = nc.dram_tensor("OW", [QBLK, S], ow_dt, kind="ExternalOutput").ap()
    OO = nc.dram_tensor("OO", [H, QBLK], f32, kind="ExternalOutput").ap()

    with ExitStack() as ctx:
        tc = ctx.enter_context(tile.TileContext(nc))
        consts = ctx.enter_context(tc.tile_pool(name="consts", bufs=1))
        big = ctx.enter_context(tc.tile_pool(name="big", bufs=1))
        ftmp = ctx.enter_context(tc.tile_pool(name="ftmp", bufs=1))
        ps = ctx.enter_context(tc.tile_pool(name="ps", bufs=4, space="PSUM"))

        # ACT table preload: first ACT op is Ln; with the patched tables one
        # load of natural_log_exp_and_others covers Ln/Exp/Square/Copy.
        junk = consts.tile([128, 1], f32)
        nc.vector.memset(junk[:], 1.0)
        junk2 = consts.tile([128, 1], f32)
        nc.scalar.activation(junk2[:], junk[:], AF.Ln)

        # --- DMA inputs ---
        kn = big.tile([128, NJT, H], f32)
        knr = K.rearrange("(t p) h -> p t h", p=128)
        nc.sync.dma_start(out=kn[:, 0:4, :], in_=knr[:, 0:4, :])
        nc.sync.dma_start(out=kn[:, 4:8, :], in_=knr[:, 4:8, :])
        idt = consts.tile([H, H], f32)
        nc.sync.dma_start(out=idt[:], in_=IDT[:])
        qn = big.tile([128, NIT, H], f32)
        nc.sync.dma_start(out=qn[:], in_=Qs.rearrange("(t p) h -> p t h", p=128))
        cdf = consts.tile([H, 2 * F], f32)
        nc.sync.dma_start(out=cdf[:], in_=CD[:])
        vn = big.tile([128, NJT, H], f32)
        nc.sync.dma_start(out=vn[:], in_=V.rearrange("(t p) h -> p t h", p=128))
        cdb = consts.tile([H, 2 * F], bf16)
        nc.vector.tensor_copy(cdb[:], cdf[:])
        vb = big.tile([128, NJT, H], bf16)
        nc.vector.tensor_copy(vb[:], vn[:])

        # --- transposes: K^T, Q^T (h on partitions) via PE ---
        kt_ps = ps.tile([128, S], f32, tag="ps")
        ktb = big.tile([128, S], bf16)
        for hh in range(2):
            for t in range(hh * 4, hh * 4 + 4):
                nc.tensor.transpose(kt_ps[:, t * 128:(t + 1) * 128],
                                    kn[:, t, :], idt[:])
            nc.scalar.copy(ktb[:, hh * 512:(hh + 1) * 512],
                           kt_ps[:, hh * 512:(hh + 1) * 512])

        qt_ps = ps.tile([128, QBLK], f32, tag="ps")
        for t in range(NIT):
            nc.tensor.transpose(qt_ps[:, t * 128:(t + 1) * 128], qn[:, t, :],
                                idt[:])
        qtb = big.tile([128, QBLK], bf16)
        nc.vector.tensor_copy(qtb[:], qt_ps[:])

        # --- DFT + features; K processed in two column halves so score
        # matmuls on half 0 overlap half 1's feature chain ---
        def dft_chunk(xtb, re_ps, im_ps, s0, c0, c1):
            nc.tensor.matmul(re_ps[:, c0:c1], cdb[:, 0:F],
                             xtb[:, s0:s0 + (c1 - c0)], start=True, stop=True)
            nc.tensor.matmul(im_ps[:, c0:c1], cdb[:, F:2 * F],
                             xtb[:, s0:s0 + (c1 - c0)], start=True, stop=True)

        def feat_chunk(re_ps, im_ps, fl, c0, c1, Ux, Vx, Wx):
            sqre, sqim, amp2, lg, tq = fl
            cc = slice(c0, c1)
            nc.scalar.activation(sqre[:, cc], re_ps[:, cc], AF.Square)
            nc.scalar.activation(sqim[:, cc], im_ps[:, cc], AF.Square)
            nc.vector.tensor_add(amp2[:, cc], sqre[:, cc], sqim[:, cc])
            nc.scalar.activation(lg[:, cc], amp2[:, cc], AF.Ln)
            nc.scalar.activation(tq[:, cc], lg[:, cc], AF.Exp, scale=-0.25)
            nc.scalar.activation(Wx[:, cc], lg[:, cc], AF.Exp, scale=0.25)
            nc.vector.tensor_mul(Ux[:, cc], re_ps[:, cc], tq[:, cc])
            nc.vector.tensor_mul(Vx[:, cc], im_ps[0:F - 1, cc],
                                 tq[0:F - 1, cc])

        uk = big.tile([F, S], bf16)
        vk = big.tile([F - 1, S], bf16)
        wk = big.tile([F, S], bf16)
        uq = big.tile([F, QBLK], bf16)
        vq = big.tile([F - 1, QBLK], bf16)
        wq = big.tile([F, QBLK], bf16)
        sqrek = ftmp.tile([F, S], bf16, tag="sqrek")
        sqimk = ftmp.tile([F, S], bf16, tag="sqimk")
        amp2k = ftmp.tile([F, S], bf16, tag="amp2k")
        lgk = ftmp.tile([F, S], f32, tag="lgk")
        tqk = ftmp.tile([F, S], f32, tag="tqk")
        flk = (sqrek, sqimk, amp2k, lgk, tqk)
        sqreq = ftmp.tile([F, QBLK], bf16, tag="sqreq")
        sqimq = ftmp.tile([F, QBLK], bf16, tag="sqimq")
        amp2q = ftmp.tile([F, QBLK], bf16, tag="amp2q")
        lgq = ftmp.tile([F, QBLK], f32, tag="lgq")
        tqq = ftmp.tile([F, QBLK], f32, tag="tqq")
        flq = (sqreq, sqimq, amp2q, lgq, tqq)
        rek = ps.tile([F, S], f32, tag="ps")
        imk = ps.tile([F, S], f32, tag="ps")
        req = ps.tile([F, QBLK], f32, tag="ps")
        imq = ps.tile([F, QBLK], f32, tag="ps")
        dft_chunk(ktb, rek, imk, 0, 0, 512)
        dft_chunk(qtb, req, imq, 0, 0, QBLK)
        feat_chunk(rek, imk, flk, 0, 512, uk, vk, wk)
        dft_chunk(ktb, rek, imk, 512, 512, 1024)
        feat_chunk(req, imq, flq, 0, QBLK, uq, vq, wq)
        feat_chunk(rek, imk, flk, 512, 1024, uk, vk, wk)

        # --- scores + softmax per query sub-tile (natural [i_p, j]) ---
        wb_tiles = []
        for it in range(NIT):
            i0, i1 = it * 128, (it + 1) * 128
            num_ps = ps.tile([128, S], f32, tag="ps")
            den_ps = ps.tile([128, S], f32, tag="ps")
            for c0 in range(0, S, 512):
                c1 = c0 + 512
                nc.tensor.matmul(num_ps[:, c0:c1], uq[:, i0:i1], uk[:, c0:c1],
                                 start=True, stop=False)
                nc.tensor.matmul(num_ps[:, c0:c1], vq[:, i0:i1], vk[:, c0:c1],
                                 start=False, stop=True)
                nc.tensor.matmul(den_ps[:, c0:c1], wq[:, i0:i1], wk[:, c0:c1],
                                 start=True, stop=True)
            inv = ftmp.tile([128, S], f32, tag="inv")
            nc.vector.reciprocal_approx_fast(out=inv[:], in_=den_ps[:])
            pa = ftmp.tile([128, S], f32, tag="pa")
            nc.vector.tensor_mul(pa[:], num_ps[:], inv[:])
            e = ftmp.tile([128, S], bf16, tag="e")
            sumexp = ftmp.tile([128, 1], f32, tag="sumexp")
            nc.scalar.activation(e[:], pa[:], AF.Exp, bias=1.0,
                                 accum_out=sumexp[:])
            r = ftmp.tile([128, 1], f32, tag="r")
            nc.vector.reciprocal(r[:], sumexp[:])
            wb = big.tile([128, S], bf16, tag=f"wb{it}")
            nc.vector.tensor_scalar_mul(wb[:], e[:], r[:])
            wb_tiles.append(wb)
            if WEIGHTS_BF16:
                nc.sync.dma_start(out=OW[i0:i1, :], in_=wb[:])
            else:
                oww = ftmp.tile([128, S], f32, tag="oww")
                nc.vector.tensor_scalar_mul(oww[:], e[:], r[:])
                nc.sync.dma_start(out=OW[i0:i1, :], in_=oww[:])

        # --- transpose normalized weights to [j_p, i] for P@V ---
        idtb = consts.tile([H, H], bf16)
        nc.vector.tensor_copy(idtb[:], idt[:])
        et_tiles = []
        for g in range(2):  # two groups of 4 j-tiles share one psum tile
            wt_ps = ps.tile([128, 4 * QBLK], bf16, tag="ps")
            for lj in range(4):
                jt = g * 4 + lj
                for it in range(NIT):
                    nc.tensor.transpose(
                        wt_ps[:, lj * QBLK + it * 128:
                              lj * QBLK + (it + 1) * 128],
                        wb_tiles[it][:, jt * 128:(jt + 1) * 128], idtb[:])
            for lj in range(4):
                jt = g * 4 + lj
                et = big.tile([128, QBLK], bf16, tag=f"et{jt}")
                eng = nc.vector.tensor_copy if (jt % 2) else nc.scalar.copy
                eng(et[:], wt_ps[:, lj * QBLK:(lj + 1) * QBLK])
                et_tiles.append(et)

        # --- P@V: out^T[h, i] accumulated over j tiles ---
        av_ps = ps.tile([128, QBLK], f32, tag="ps")
        for jt in range(NJT):
            nc.tensor.matmul(av_ps[:], vb[:, jt, :], et_tiles[jt][:],
                             start=(jt == 0), stop=(jt == NJT - 1))
        oo = big.tile([H, QBLK], f32)
        nc.vector.tensor_copy(oo[:], av_ps[:])
        nc.sync.dma_start(out=OO[:], in_=oo[:])

    nc.compile()
    return nc


def _get_nc():
    if "nc" not in _CACHE:
        _CACHE["nc"] = _build()
    return _CACHE["nc"]


def kernel(Q, K, V):
    from concourse.bass_utils import run_bass_kernel_spmd

    Q = np.ascontiguousarray(np.asarray(Q, dtype=np.float32))
    K = np.ascontiguousarray(np.asarray(K, dtype=np.float32))
    V = np.ascontiguousarray(np.asarray(V, dtype=np.float32))
    nc = _get_nc()
    in_maps = []
    for c in range(NCORES):
        b, qb = c // 4, c % 4
        in_maps.append({
            "Qs": np.ascontiguousarray(Q[b, qb * QBLK:(qb + 1) * QBLK]),
            "K": K[b],
            "V": V[b],
        })
    res = run_bass_kernel_spmd(nc, in_maps, core_ids=list(range(NCORES)))
    output = np.empty((B, S, H), np.float32)
    weights = np.empty((B, S, S), np.float32)
    for c in range(NCORES):
        b, qb = c // 4, c % 4
        rr = res.results[c]
        weights[b, qb * QBLK:(qb + 1) * QBLK, :] = rr["OW"].astype(np.float32)
        output[b, qb * QBLK:(qb + 1) * QBLK, :] = rr["OO"].T
    return output, weights


if __name__ == "__main__":
    rng = np.random.default_rng(0)
    Q = rng.standard_normal((B, S, H)).astype(np.float32)
    K = rng.standard_normal((B, S, H)).astype(np.float32)
    V = rng.standard_normal((B, S, H)).astype(np.float32)
    out, w = kernel(Q, K, V)
    print("kernel ran:", out.shape, w.shape)
